# revision 13
# baseline (speedup 1.0000x reference)
"""MultiHeadExternalAttention Trainium2 kernel.

Math (reference):
  h = x @ trans_w.T + trans_b            [B,N,4096] -> heads [B,64,N,64]
  a = h @ lin0_w.T + lin0_b              per-head [B,64,N,64]
  a = softmax(a, axis=N)
  a = a / (1e-10 + a.sum(-1, keepdims))  double norm over j
  o = a @ lin1_w.T + lin1_b
  out = o (merged heads) @ proj_w.T + proj_b

h is only consumed through lin0, and lin1 feeds straight into proj, so both
tiny linears fold into the big matmuls on the host:
  logits[b,h,n,j] = x[b,n,:] @ fw[h,j,:] + fb[h,j]     fw = lin0_w @ trans_w_h
  out[b,n,c]     += attn[b,h,n,k] * g[h,c,k]           g  = proj_w_h @ lin1_w
  out[b,n,c]     += cb[c]                               (host)

Sharding: 8 cores = 4 batches x 2 head-halves (32 heads = 16 head-pairs per
core). Per (core, head-pair): logits computed in [j2=128, n] layout (j2 = two
heads' 64 lin0-outputs stacked) so softmax over n is a free-dim op; attn is
written back through a j-sum matmul (shifted-mask trick -> [2w+g, n] psum
rows), reciprocal, broadcast matmul, and one fused scalar_tensor_tensor.
Normalized attn (bf16) goes to a DRAM scratch; phase 2 re-reads it chunk-wise
as matmul lhsT against the folded projection weights, accumulating all 16
head-pairs in PSUM. Host sums the two head-half partials per batch + bias.
"""

import sys

if "/opt/trn_rl_repo" not in sys.path:
    sys.path.insert(0, "/opt/trn_rl_repo")

import numpy as np
import ml_dtypes

import concourse.bass as bass
import concourse.bacc as bacc
import concourse.mybir as mybir
import concourse.tile as tile
from concourse.masks import make_identity

BF16NP = ml_dtypes.bfloat16
F32 = mybir.dt.float32
F32R = mybir.dt.float32r
BF = mybir.dt.bfloat16
AF = mybir.ActivationFunctionType

DIM = 512
HEADS = 64
K = 64
B = 4
NTOK = 8192
NCORES = 8
HPC = 16  # head pairs per core


def build_bass(ntok=NTOK, n_hp=HPC):
    nc = bacc.Bacc()
    x_in = nc.dram_tensor("x_in", [ntok, DIM], F32, kind="ExternalInput")
    w2 = nc.dram_tensor("w2", [128, n_hp, 4, 128], BF, kind="ExternalInput")
    c1 = nc.dram_tensor("c1", [128, n_hp], F32, kind="ExternalInput")
    p2 = nc.dram_tensor("p2", [128, n_hp, DIM], BF, kind="ExternalInput")
    d2 = nc.dram_tensor("d2", [2, 128], BF, kind="ExternalInput")
    out_p = nc.dram_tensor("out_p", [ntok, DIM], F32, kind="ExternalOutput")

    NT = ntok // 128
    NW = ntok // 512
    NCH = ntok // 1024

    with tile.TileContext(nc) as tc:
        with tc.tile_pool(name="const", bufs=1) as const, tc.tile_pool(
            name="dramp", bufs=1, space="DRAM"
        ) as dramp:
            scratch = dramp.tile([n_hp, 128, ntok], BF)

            w2_sb = const.tile([128, n_hp, 4, 128], BF)
            nc.sync.dma_start(out=w2_sb, in_=w2[:])
            c1_sb = const.tile([128, n_hp], F32)
            nc.sync.dma_start(out=c1_sb, in_=c1[:])
            p2_sb = const.tile([128, n_hp, DIM], BF)
            nc.sync.dma_start(out=p2_sb, in_=p2[:])

            xT = [const.tile([128, ntok], BF, name=f"xT{i}") for i in range(4)]
            # Tm: jsum mask. Column 32 rows 0:64 / column 33 rows 64:128 hold
            # 1/s per partition; the window-w matmul view Tm[:, 32-2w:64-2w]
            # places them at free positions 2w, 2w+1 -> psum rows 2w+g.
            Tm = const.tile([128, 66], BF)
            nc.vector.memset(Tm, 0.0)
            # D2: broadcast mask, ones block per head group (host constant).
            D2 = const.tile([2, 128], BF)
            nc.sync.dma_start(out=D2, in_=d2[:])

            ident = const.tile([128, 128], F32)
            make_identity(nc, ident)

            # ---- x load + PE-transpose + cast into xT[cc][c, n] ----
            with tc.tile_pool(name="xload", bufs=3) as xload, tc.tile_pool(
                name="xtp", bufs=4, space="PSUM"
            ) as xtp:
                for t in range(NT):
                    xf = xload.tile([128, DIM], F32, name="xf")
                    nc.sync.dma_start(out=xf, in_=x_in[128 * t : 128 * (t + 1), :])
                    for cc in range(4):
                        xps = xtp.tile([128, 128], F32, name="xps")
                        nc.tensor.transpose(
                            xps, xf[:, 128 * cc : 128 * (cc + 1)], ident
                        )
                        nc.scalar.activation(
                            xT[cc][:, 128 * t : 128 * (t + 1)], xps, func=AF.Copy
                        )

            # ---- phase 1: per head-pair softmax pipeline ----
            with tc.tile_pool(name="p1p", bufs=2, space="PSUM") as p1p, tc.tile_pool(
                name="jsp", bufs=2, space="PSUM"
            ) as jsp, tc.tile_pool(
                name="scp", bufs=2, space="PSUM"
            ) as scp, tc.tile_pool(name="ep", bufs=2) as ep, tc.tile_pool(
                name="anp", bufs=2
            ) as anp, tc.tile_pool(name="rp", bufs=2) as rp, tc.tile_pool(
                name="small", bufs=2
            ) as small:
                for hp in range(n_hp):
                    e = ep.tile([128, ntok], BF, name="e")
                    scol = small.tile([128, NCH], F32, name="scol")
                    for t8 in range(NCH):
                        p1 = p1p.tile([128, 1024], F32, name="p1")
                        for half in range(2):
                            base = 1024 * t8 + 512 * half
                            for cc in range(4):
                                nc.tensor.matmul(
                                    p1[:, 512 * half : 512 * (half + 1)],
                                    lhsT=w2_sb[:, hp, cc, :],
                                    rhs=xT[cc][:, base : base + 512],
                                    start=(cc == 0),
                                    stop=(cc == 3),
                                )
                        nc.scalar.activation(
                            e[:, 1024 * t8 : 1024 * (t8 + 1)],
                            p1,
                            func=AF.Exp,
                            bias=c1_sb[:, hp : hp + 1],
                            scale=1.0,
                            accum_out=scol[:, t8 : t8 + 1],
                        )
                    s1 = small.tile([128, 1], F32, name="s1")
                    nc.vector.reduce_sum(s1, scol, axis=mybir.AxisListType.X)
                    rs = small.tile([128, 1], F32, name="rs")
                    nc.vector.reciprocal(rs, s1)
                    nc.vector.tensor_copy(Tm[0:64, 32:33], rs[0:64, :])
                    nc.vector.tensor_copy(Tm[64:128, 33:34], rs[64:128, :])

                    js = jsp.tile([32, 512], F32, name="js")
                    for w in range(NW):
                        nc.tensor.matmul(
                            js,
                            lhsT=Tm[:, 32 - 2 * w : 64 - 2 * w],
                            rhs=e[:, 512 * w : 512 * (w + 1)],
                            start=(w == 0),
                            stop=(w == NW - 1),
                        )
                    r32 = rp.tile([32, 512], F32, name="r32")
                    nc.vector.reciprocal(r32[0 : 2 * NW, :], js[0 : 2 * NW, :])
                    r32b = rp.tile([32, 512], BF, name="r32b")
                    nc.vector.tensor_copy(r32b[0 : 2 * NW, :], r32[0 : 2 * NW, :])

                    attn = anp.tile([128, ntok], BF, name="attn")
                    for w in range(NW):
                        r2w = rp.tile([2, 512], BF, name="r2w")
                        nc.sync.dma_start(out=r2w, in_=r32b[2 * w : 2 * w + 2, :])
                        sc = scp.tile([128, 512], F32, name="sc")
                        nc.tensor.matmul(
                            sc,
                            lhsT=D2,
                            rhs=r2w,
                            start=True,
                            stop=True,
                        )
                        nc.vector.scalar_tensor_tensor(
                            out=attn[:, 512 * w : 512 * (w + 1)],
                            in0=e[:, 512 * w : 512 * (w + 1)],
                            scalar=rs,
                            in1=sc,
                            op0=mybir.AluOpType.mult,
                            op1=mybir.AluOpType.mult,
                        )
                    nc.sync.dma_start(out=scratch[hp], in_=attn)

            # ---- phase 2: out[n,c] = sum_hp attn_hp[:, chunk].T @ p2_hp ----
            with tc.tile_pool(name="a2p", bufs=3) as a2p, tc.tile_pool(
                name="op", bufs=4, space="PSUM"
            ) as op, tc.tile_pool(name="osp", bufs=3) as osp:
                for i in range(NT):
                    a2 = a2p.tile([128, n_hp, 128], BF, name="a2")
                    for hpi in range(n_hp):
                        nc.sync.dma_start(
                            out=a2[:, hpi, :],
                            in_=scratch[hpi, :, 128 * i : 128 * (i + 1)],
                        )
                    po = op.tile([128, 512], F32, name="po")
                    for hpi in range(n_hp):
                        nc.tensor.matmul(
                            po,
                            lhsT=a2[:, hpi, :],
                            rhs=p2_sb[:, hpi, :],
                            start=(hpi == 0),
                            stop=(hpi == n_hp - 1),
                        )
                    osb = osp.tile([128, DIM], F32, name="osb")
                    nc.scalar.activation(osb, po, func=AF.Copy)
                    nc.sync.dma_start(out=out_p[128 * i : 128 * (i + 1), :], in_=osb)
    nc.finalize()
    return nc


def fuse_weights(inputs):
    tw = np.asarray(inputs["trans_w"], np.float64)  # [4096, 512]
    tb = np.asarray(inputs["trans_b"], np.float64)  # [4096]
    l0w = np.asarray(inputs["lin0_w"], np.float64)  # [64, 64]
    l0b = np.asarray(inputs["lin0_b"], np.float64)
    l1w = np.asarray(inputs["lin1_w"], np.float64)
    l1b = np.asarray(inputs["lin1_b"], np.float64)
    pw = np.asarray(inputs["proj_w"], np.float64)  # [512, 4096]
    pb = np.asarray(inputs["proj_b"], np.float64)

    tw3 = tw.reshape(HEADS, K, DIM)
    tb2 = tb.reshape(HEADS, K)
    fw = np.einsum("jk,hkc->hjc", l0w, tw3)  # [64, 64, 512]
    fb = l0b[None, :] + np.einsum("jk,hk->hj", l0w, tb2)  # [64, 64]
    pw3 = pw.reshape(DIM, HEADS, K).transpose(1, 0, 2)  # [h, c, j]
    g = np.einsum("hcj,jk->hck", pw3, l1w)  # [64, 512, 64]
    cb = pb + np.einsum("hcj,j->c", pw3, l1b)  # [512]
    return fw, fb, g, cb


def make_core_inputs(x, fw, fb, g, b, gg, n_hp=HPC):
    """Inputs for the core handling batch b, head half gg (heads 32*gg..+32)."""
    ntok = x.shape[1]
    h0 = (HEADS // 2) * gg
    w2 = np.empty((128, n_hp, 4, 128), BF16NP)
    c1 = np.empty((128, n_hp), np.float32)
    p2 = np.empty((128, n_hp, DIM), BF16NP)
    for hp in range(n_hp):
        ha, hb = h0 + 2 * hp, h0 + 2 * hp + 1
        blk = np.concatenate([fw[ha], fw[hb]], axis=0)  # [128 j2, 512 c]
        # w2[ci, hp, cc, j2] = blk[j2, cc*128+ci]
        w2[:, hp, :, :] = blk.reshape(128, 4, 128).transpose(2, 1, 0).astype(BF16NP)
        c1[:, hp] = np.concatenate([fb[ha], fb[hb]]).astype(np.float32)
        # p2[g2*64+k, hp, c] = g[head, c, k]
        p2[0:64, hp, :] = g[ha].T.astype(BF16NP)
        p2[64:128, hp, :] = g[hb].T.astype(BF16NP)
    d2 = np.zeros((2, 128), BF16NP)
    d2[0, 0:64] = 1.0
    d2[1, 64:128] = 1.0
    return {
        "x_in": np.ascontiguousarray(x[b], np.float32),
        "w2": w2,
        "c1": c1,
        "p2": p2,
        "d2": d2,
    }


_NC_CACHE = None
LAST_RESULTS = None


def kernel(**inputs):
    global _NC_CACHE, LAST_RESULTS
    import os
    from concourse.bass_utils import run_bass_kernel_spmd

    x = np.asarray(inputs["x"], np.float32)
    fw, fb, g, cb = fuse_weights(inputs)

    if _NC_CACHE is None:
        _NC_CACHE = build_bass()
    nc = _NC_CACHE

    in_maps = []
    for c in range(NCORES):
        b, gg = c // 2, c % 2
        in_maps.append(make_core_inputs(x, fw, fb, g, b, gg))

    trace = os.environ.get("MEA_TRACE", "0") == "1"
    res = run_bass_kernel_spmd(nc, in_maps, list(range(NCORES)), trace=trace)
    LAST_RESULTS = res

    out = np.empty((B, NTOK, DIM), np.float32)
    cbf = cb.astype(np.float32)
    for b in range(B):
        out[b] = res.results[2 * b]["out_p"] + res.results[2 * b + 1]["out_p"]
        out[b] += cbf[None, :]
    return out


# revision 16
# speedup vs baseline: 63.1592x; 63.1592x over previous
"""MultiHeadExternalAttention Trainium2 kernel.

Math (reference):
  h = x @ trans_w.T + trans_b            [B,N,4096] -> heads [B,64,N,64]
  a = h @ lin0_w.T + lin0_b              per-head [B,64,N,64]
  a = softmax(a, axis=N)
  a = a / (1e-10 + a.sum(-1, keepdims))  double norm over j
  o = a @ lin1_w.T + lin1_b
  out = o (merged heads) @ proj_w.T + proj_b

h is only consumed through lin0, and lin1 feeds straight into proj, so both
tiny linears fold into the big matmuls on the host:
  logits[b,h,n,j] = x[b,n,:] @ fw[h,j,:] + fb[h,j]     fw = lin0_w @ trans_w_h
  out[b,n,c]     += attn[b,h,n,k] * g[h,c,k]           g  = proj_w_h @ lin1_w
  out[b,n,c]     += cb[c]                               (host)

Sharding: 8 cores = 4 batches x 2 head-halves (32 heads = 16 head-pairs per
core). Per (core, head-pair): logits computed in [j2=128, n] layout (j2 = two
heads' 64 lin0-outputs stacked) so softmax over n is a free-dim op; attn is
written back through a j-sum matmul (shifted-mask trick -> [2w+g, n] psum
rows), reciprocal, broadcast matmul, and one fused scalar_tensor_tensor.
Normalized attn (bf16) goes to a DRAM scratch; phase 2 re-reads it chunk-wise
as matmul lhsT against the folded projection weights, accumulating all 16
head-pairs in PSUM. Host sums the two head-half partials per batch + bias.
"""

import sys

if "/opt/trn_rl_repo" not in sys.path:
    sys.path.insert(0, "/opt/trn_rl_repo")

import numpy as np
import ml_dtypes

import concourse.bass as bass
import concourse.bacc as bacc
import concourse.mybir as mybir
import concourse.tile as tile
from concourse.masks import make_identity

BF16NP = ml_dtypes.bfloat16
F32 = mybir.dt.float32
F32R = mybir.dt.float32r
BF = mybir.dt.bfloat16
AF = mybir.ActivationFunctionType

DIM = 512
HEADS = 64
K = 64
B = 4
NTOK = 8192
NCORES = 8
HPC = 16  # head pairs per core


def build_bass(ntok=NTOK, n_hp=HPC, reps=1):
    nc = bacc.Bacc()
    x_in = nc.dram_tensor("x_in", [ntok, DIM], F32, kind="ExternalInput")
    w2 = nc.dram_tensor("w2", [128, n_hp, 4, 128], BF, kind="ExternalInput")
    c1 = nc.dram_tensor("c1", [128, n_hp], F32, kind="ExternalInput")
    p2 = nc.dram_tensor("p2", [128, n_hp, DIM], BF, kind="ExternalInput")
    d2 = nc.dram_tensor("d2", [2, 128], BF, kind="ExternalInput")
    out_p = nc.dram_tensor("out_p", [ntok, DIM], F32, kind="ExternalOutput")

    NT = ntok // 128
    NW = ntok // 512
    NCH = ntok // 1024

    with tile.TileContext(nc) as tc:
        with tc.tile_pool(name="const", bufs=1) as const, tc.tile_pool(
            name="dramp", bufs=1, space="DRAM"
        ) as dramp:
            scratch = dramp.tile([n_hp, 128, ntok], BF)

            w2_sb = const.tile([128, n_hp, 4, 128], BF)
            nc.sync.dma_start(out=w2_sb, in_=w2[:])
            c1_sb = const.tile([128, n_hp], F32)
            nc.sync.dma_start(out=c1_sb, in_=c1[:])
            p2_sb = const.tile([128, n_hp, DIM], BF)
            nc.sync.dma_start(out=p2_sb, in_=p2[:])

            xT = [const.tile([128, ntok], BF, name=f"xT{i}") for i in range(4)]
            # Tm: jsum mask. Column 32 rows 0:64 / column 33 rows 64:128 hold
            # 1/s per partition; the window-w matmul view Tm[:, 32-2w:64-2w]
            # places them at free positions 2w, 2w+1 -> psum rows 2w+g.
            Tm = const.tile([128, 66], BF)
            nc.vector.memset(Tm, 0.0)
            # D2: broadcast mask, ones block per head group (host constant).
            D2 = const.tile([2, 128], BF)
            nc.sync.dma_start(out=D2, in_=d2[:])

            ident = const.tile([128, 128], F32)
            make_identity(nc, ident)

            for _rep in range(reps):
                run_pipeline(
                    nc, tc, x_in, out_p, scratch, w2_sb, c1_sb, p2_sb, xT, Tm, D2,
                    ident, ntok, n_hp,
                )
    nc.finalize()
    return nc


def run_pipeline(
    nc, tc, x_in, out_p, scratch, w2_sb, c1_sb, p2_sb, xT, Tm, D2, ident, ntok, n_hp
):
    NT = ntok // 128
    NW = ntok // 512
    NCH = ntok // 1024
    if True:
        if True:
            # ---- x load + PE-transpose + cast into xT[cc][c, n] ----
            with tc.tile_pool(name="xload", bufs=3) as xload, tc.tile_pool(
                name="xtp", bufs=4, space="PSUM"
            ) as xtp:
                for t in range(NT):
                    xf = xload.tile([128, DIM], F32, name="xf")
                    nc.sync.dma_start(out=xf, in_=x_in[128 * t : 128 * (t + 1), :])
                    for cc in range(4):
                        xps = xtp.tile([128, 128], F32, name="xps")
                        nc.tensor.transpose(
                            xps, xf[:, 128 * cc : 128 * (cc + 1)], ident
                        )
                        nc.scalar.activation(
                            xT[cc][:, 128 * t : 128 * (t + 1)], xps, func=AF.Copy
                        )

            # ---- phase 1: per head-pair softmax pipeline ----
            with tc.tile_pool(name="p1p", bufs=2, space="PSUM") as p1p, tc.tile_pool(
                name="jsp", bufs=2, space="PSUM"
            ) as jsp, tc.tile_pool(
                name="scp", bufs=2, space="PSUM"
            ) as scp, tc.tile_pool(name="ep", bufs=2) as ep, tc.tile_pool(
                name="anp", bufs=2
            ) as anp, tc.tile_pool(name="rp", bufs=2) as rp, tc.tile_pool(
                name="small", bufs=2
            ) as small:
                for hp in range(n_hp):
                    e = ep.tile([128, ntok], BF, name="e")
                    scol = small.tile([128, NCH], F32, name="scol")
                    for t8 in range(NCH):
                        p1 = p1p.tile([128, 1024], F32, name="p1")
                        for half in range(2):
                            base = 1024 * t8 + 512 * half
                            for cc in range(4):
                                nc.tensor.matmul(
                                    p1[:, 512 * half : 512 * (half + 1)],
                                    lhsT=w2_sb[:, hp, cc, :],
                                    rhs=xT[cc][:, base : base + 512],
                                    start=(cc == 0),
                                    stop=(cc == 3),
                                )
                        nc.scalar.activation(
                            e[:, 1024 * t8 : 1024 * (t8 + 1)],
                            p1,
                            func=AF.Exp,
                            bias=c1_sb[:, hp : hp + 1],
                            scale=1.0,
                            accum_out=scol[:, t8 : t8 + 1],
                        )
                    s1 = small.tile([128, 1], F32, name="s1")
                    nc.vector.reduce_sum(s1, scol, axis=mybir.AxisListType.X)
                    rs = small.tile([128, 1], F32, name="rs")
                    nc.vector.reciprocal(rs, s1)
                    nc.vector.tensor_copy(Tm[0:64, 32:33], rs[0:64, :])
                    nc.vector.tensor_copy(Tm[64:128, 33:34], rs[64:128, :])

                    js = jsp.tile([32, 512], F32, name="js")
                    for w in range(NW):
                        nc.tensor.matmul(
                            js,
                            lhsT=Tm[:, 32 - 2 * w : 64 - 2 * w],
                            rhs=e[:, 512 * w : 512 * (w + 1)],
                            start=(w == 0),
                            stop=(w == NW - 1),
                        )
                    r32 = rp.tile([32, 512], F32, name="r32")
                    nc.vector.reciprocal(r32[0 : 2 * NW, :], js[0 : 2 * NW, :])
                    r32b = rp.tile([32, 512], BF, name="r32b")
                    nc.vector.tensor_copy(r32b[0 : 2 * NW, :], r32[0 : 2 * NW, :])

                    attn = anp.tile([128, ntok], BF, name="attn")
                    for w in range(NW):
                        r2w = rp.tile([2, 512], BF, name="r2w")
                        nc.sync.dma_start(out=r2w, in_=r32b[2 * w : 2 * w + 2, :])
                        sc = scp.tile([128, 512], F32, name="sc")
                        nc.tensor.matmul(
                            sc,
                            lhsT=D2,
                            rhs=r2w,
                            start=True,
                            stop=True,
                        )
                        nc.vector.scalar_tensor_tensor(
                            out=attn[:, 512 * w : 512 * (w + 1)],
                            in0=e[:, 512 * w : 512 * (w + 1)],
                            scalar=rs,
                            in1=sc,
                            op0=mybir.AluOpType.mult,
                            op1=mybir.AluOpType.mult,
                        )
                    nc.sync.dma_start(out=scratch[hp], in_=attn)

            # ---- phase 2: out[n,c] = sum_hp attn_hp[:, chunk].T @ p2_hp ----
            with tc.tile_pool(name="a2p", bufs=3) as a2p, tc.tile_pool(
                name="op", bufs=4, space="PSUM"
            ) as op, tc.tile_pool(name="osp", bufs=3) as osp:
                for i in range(NT):
                    a2 = a2p.tile([128, n_hp, 128], BF, name="a2")
                    for hpi in range(n_hp):
                        nc.sync.dma_start(
                            out=a2[:, hpi, :],
                            in_=scratch[hpi, :, 128 * i : 128 * (i + 1)],
                        )
                    po = op.tile([128, 512], F32, name="po")
                    for hpi in range(n_hp):
                        nc.tensor.matmul(
                            po,
                            lhsT=a2[:, hpi, :],
                            rhs=p2_sb[:, hpi, :],
                            start=(hpi == 0),
                            stop=(hpi == n_hp - 1),
                        )
                    osb = osp.tile([128, DIM], F32, name="osb")
                    nc.scalar.activation(osb, po, func=AF.Copy)
                    nc.sync.dma_start(out=out_p[128 * i : 128 * (i + 1), :], in_=osb)


def fuse_weights(inputs):
    tw = np.asarray(inputs["trans_w"], np.float64)  # [4096, 512]
    tb = np.asarray(inputs["trans_b"], np.float64)  # [4096]
    l0w = np.asarray(inputs["lin0_w"], np.float64)  # [64, 64]
    l0b = np.asarray(inputs["lin0_b"], np.float64)
    l1w = np.asarray(inputs["lin1_w"], np.float64)
    l1b = np.asarray(inputs["lin1_b"], np.float64)
    pw = np.asarray(inputs["proj_w"], np.float64)  # [512, 4096]
    pb = np.asarray(inputs["proj_b"], np.float64)

    tw3 = tw.reshape(HEADS, K, DIM)
    tb2 = tb.reshape(HEADS, K)
    fw = np.einsum("jk,hkc->hjc", l0w, tw3)  # [64, 64, 512]
    fb = l0b[None, :] + np.einsum("jk,hk->hj", l0w, tb2)  # [64, 64]
    pw3 = pw.reshape(DIM, HEADS, K).transpose(1, 0, 2)  # [h, c, j]
    g = np.einsum("hcj,jk->hck", pw3, l1w)  # [64, 512, 64]
    cb = pb + np.einsum("hcj,j->c", pw3, l1b)  # [512]
    return fw, fb, g, cb


def make_core_inputs(x, fw, fb, g, b, gg, n_hp=HPC):
    """Inputs for the core handling batch b, head half gg (heads 32*gg..+32)."""
    ntok = x.shape[1]
    h0 = (HEADS // 2) * gg
    w2 = np.empty((128, n_hp, 4, 128), BF16NP)
    c1 = np.empty((128, n_hp), np.float32)
    p2 = np.empty((128, n_hp, DIM), BF16NP)
    for hp in range(n_hp):
        ha, hb = h0 + 2 * hp, h0 + 2 * hp + 1
        blk = np.concatenate([fw[ha], fw[hb]], axis=0)  # [128 j2, 512 c]
        # w2[ci, hp, cc, j2] = blk[j2, cc*128+ci]
        w2[:, hp, :, :] = blk.reshape(128, 4, 128).transpose(2, 1, 0).astype(BF16NP)
        c1[:, hp] = np.concatenate([fb[ha], fb[hb]]).astype(np.float32)
        # p2[g2*64+k, hp, c] = g[head, c, k]
        p2[0:64, hp, :] = g[ha].T.astype(BF16NP)
        p2[64:128, hp, :] = g[hb].T.astype(BF16NP)
    d2 = np.zeros((2, 128), BF16NP)
    d2[0, 0:64] = 1.0
    d2[1, 64:128] = 1.0
    return {
        "x_in": np.ascontiguousarray(x[b], np.float32),
        "w2": w2,
        "c1": c1,
        "p2": p2,
        "d2": d2,
    }


_NC_CACHE = None
LAST_RESULTS = None


def kernel(**inputs):
    global _NC_CACHE, LAST_RESULTS
    import os
    from concourse.bass_utils import run_bass_kernel_spmd

    x = np.asarray(inputs["x"], np.float32)
    fw, fb, g, cb = fuse_weights(inputs)

    if _NC_CACHE is None:
        _NC_CACHE = build_bass()
    nc = _NC_CACHE

    in_maps = []
    for c in range(NCORES):
        b, gg = c // 2, c % 2
        in_maps.append(make_core_inputs(x, fw, fb, g, b, gg))

    trace = os.environ.get("MEA_TRACE", "0") == "1"
    res = run_bass_kernel_spmd(nc, in_maps, list(range(NCORES)), trace=trace)
    LAST_RESULTS = res

    out = np.empty((B, NTOK, DIM), np.float32)
    cbf = cb.astype(np.float32)
    for b in range(B):
        out[b] = res.results[2 * b]["out_p"] + res.results[2 * b + 1]["out_p"]
        out[b] += cbf[None, :]
    return out


# revision 34
# speedup vs baseline: 122.3734x; 1.9375x over previous
"""MultiHeadExternalAttention Trainium2 kernel.

Math (reference):
  h = x @ trans_w.T + trans_b            [B,N,4096] -> heads [B,64,N,64]
  a = h @ lin0_w.T + lin0_b              per-head [B,64,N,64]
  a = softmax(a, axis=N)
  a = a / (1e-10 + a.sum(-1, keepdims))  double norm over j
  o = a @ lin1_w.T + lin1_b
  out = o (merged heads) @ proj_w.T + proj_b

h is only consumed through lin0, and lin1 feeds straight into proj, so both
tiny linears fold into the big matmuls on the host:
  logits[b,h,n,j] = x[b,n,:] @ fw[h,j,:] + fb[h,j]     fw = lin0_w @ trans_w_h
  out[b,n,c]     += attn[b,h,n,k] * g[h,c,k]           g  = proj_w_h @ lin1_w
  out[b,n,c]     += cb[c]                               (host)

Sharding: 8 cores = 4 batches x 2 head-halves (32 heads = 16 head-pairs per
core). Per (core, head-pair): logits computed in [j2=128, n] layout (j2 = two
heads' 64 lin0-outputs stacked) so softmax over n is a free-dim op; attn is
written back through a j-sum matmul (shifted-mask trick -> [2w+g, n] psum
rows), reciprocal, broadcast matmul, and one fused scalar_tensor_tensor.
Normalized attn (bf16) goes to a DRAM scratch; phase 2 re-reads it chunk-wise
as matmul lhsT against the folded projection weights, accumulating all 16
head-pairs in PSUM. Host sums the two head-half partials per batch + bias.
"""

import sys

if "/opt/trn_rl_repo" not in sys.path:
    sys.path.insert(0, "/opt/trn_rl_repo")

import numpy as np
import ml_dtypes

import concourse.bass as bass
import concourse.bacc as bacc
import concourse.mybir as mybir
import concourse.tile as tile
from concourse.masks import make_identity

BF16NP = ml_dtypes.bfloat16
F32 = mybir.dt.float32
F32R = mybir.dt.float32r
BF = mybir.dt.bfloat16
AF = mybir.ActivationFunctionType

DIM = 512
HEADS = 64
K = 64
B = 4
NTOK = 8192
NCORES = 8
HPC = 16  # head pairs per core


def build_bass(ntok=NTOK, n_hp=HPC, reps=1):
    nc = bacc.Bacc()
    x_in = nc.dram_tensor("x_in", [ntok, DIM], F32, kind="ExternalInput")
    w2 = nc.dram_tensor("w2", [128, n_hp, 4, 128], BF, kind="ExternalInput")
    c1 = nc.dram_tensor("c1", [128, n_hp], F32, kind="ExternalInput")
    p2 = nc.dram_tensor("p2", [128, n_hp, DIM], BF, kind="ExternalInput")
    d2 = nc.dram_tensor("d2", [2, 128], BF, kind="ExternalInput")
    out_p = nc.dram_tensor("out_p", [ntok, DIM], F32, kind="ExternalOutput")

    NT = ntok // 128
    NW = ntok // 512
    NCH = ntok // 1024

    with tile.TileContext(nc) as tc:
        with tc.tile_pool(name="const", bufs=1) as const, tc.tile_pool(
            name="dramp", bufs=1, space="DRAM"
        ) as dramp:
            scratch = dramp.tile([n_hp, 128, ntok], BF)

            w2_sb = const.tile([128, n_hp, 4, 128], BF)
            nc.sync.dma_start(out=w2_sb, in_=w2[:])
            c1_sb = const.tile([128, n_hp], F32)
            nc.sync.dma_start(out=c1_sb, in_=c1[:])
            p2_sb = const.tile([128, n_hp, DIM], BF)
            nc.sync.dma_start(out=p2_sb, in_=p2[:])

            xT = [const.tile([128, ntok], BF, name=f"xT{i}") for i in range(4)]
            # D2: broadcast mask, ones block per head group (host constant).
            D2 = const.tile([2, 128], BF)
            nc.sync.dma_start(out=D2, in_=d2[:])

            ident = const.tile([128, 128], F32)
            make_identity(nc, ident)

            for _rep in range(reps):
                run_pipeline(
                    nc, tc, x_in, out_p, scratch, w2_sb, c1_sb, p2_sb, xT, D2,
                    ident, dramp, ntok, n_hp,
                )
    nc.finalize()
    return nc


def run_pipeline(
    nc, tc, x_in, out_p, scratch, w2_sb, c1_sb, p2_sb, xT, D2, ident, dramp, ntok,
    n_hp,
):
    NT = ntok // 128
    NW = ntok // 512
    NCH = ntok // 1024
    if True:
        if True:
            # ---- x load + PE-transpose + cast into xT[cc][c, n] ----
            with tc.tile_pool(name="xload", bufs=3) as xload, tc.tile_pool(
                name="xtp", bufs=4, space="PSUM"
            ) as xtp:
                for t4 in range(NT // 4):
                    xf = xload.tile([128, 4, DIM], F32, name="xf")
                    nc.sync.dma_start(
                        out=xf,
                        in_=x_in[512 * t4 : 512 * (t4 + 1), :].rearrange(
                            "(a p) c -> p a c", p=128
                        ),
                    )
                    for a in range(4):
                        t = 4 * t4 + a
                        for cc in range(4):
                            xps = xtp.tile([128, 128], F32, name="xps")
                            nc.tensor.transpose(
                                xps, xf[:, a, 128 * cc : 128 * (cc + 1)], ident
                            )
                            nc.scalar.activation(
                                xT[cc][:, 128 * t : 128 * (t + 1)], xps, func=AF.Copy
                            )

            # ---- phase 1: per head-pair softmax pipeline (sw-pipelined) ----
            with tc.tile_pool(name="p1p", bufs=4, space="PSUM") as p1p, tc.tile_pool(
                name="jsp", bufs=2, space="PSUM"
            ) as jsp, tc.tile_pool(
                name="scp", bufs=2, space="PSUM"
            ) as scp, tc.tile_pool(name="ep", bufs=3) as ep, tc.tile_pool(
                name="anp", bufs=2
            ) as anp, tc.tile_pool(name="rp", bufs=2) as rp, tc.tile_pool(
                name="tmp", bufs=2
            ) as tmp, tc.tile_pool(name="small", bufs=3) as small:
                state = {}

                def emit_step1(hp):
                    e = ep.tile([128, ntok], BF, name="e")
                    scol = small.tile([128, NW], F32, name="scol")
                    for t8 in range(NW):
                        p1 = p1p.tile([128, 512], F32, name="p1")
                        base = 512 * t8
                        for cc in range(4):
                            nc.tensor.matmul(
                                p1,
                                lhsT=w2_sb[:, hp, cc, :],
                                rhs=xT[cc][:, base : base + 512],
                                start=(cc == 0),
                                stop=(cc == 3),
                            )
                        nc.scalar.activation(
                            e[:, base : base + 512],
                            p1,
                            func=AF.Exp,
                            bias=c1_sb[:, hp : hp + 1],
                            scale=1.0,
                            accum_out=scol[:, t8 : t8 + 1],
                        )
                    s1 = small.tile([128, 1], F32, name="s1")
                    nc.vector.reduce_sum(s1, scol, axis=mybir.AxisListType.X)
                    rs = small.tile([128, 1], F32, name="rs")
                    nc.vector.reciprocal(rs, s1)
                    Tmh = tmp.tile([128, 66], BF, name="Tmh")
                    nc.vector.memset(Tmh, 0.0)
                    nc.vector.tensor_copy(Tmh[0:64, 32:33], rs[0:64, :])
                    nc.vector.tensor_copy(Tmh[64:128, 33:34], rs[64:128, :])
                    state[hp] = (e, rs, Tmh)

                def emit_jsum(hp):
                    e, rs, Tmh = state[hp]
                    js = jsp.tile([32, 512], F32, name="js")
                    for w in range(NW):
                        nc.tensor.matmul(
                            js,
                            lhsT=Tmh[:, 32 - 2 * w : 64 - 2 * w],
                            rhs=e[:, 512 * w : 512 * (w + 1)],
                            start=(w == 0),
                            stop=(w == NW - 1),
                        )
                    r32 = rp.tile([32, 512], F32, name="r32")
                    nc.vector.reciprocal(r32[0 : 2 * NW, :], js[0 : 2 * NW, :])
                    r32b = rp.tile([32, 512], BF, name="r32b")
                    nc.vector.tensor_copy(r32b[0 : 2 * NW, :], r32[0 : 2 * NW, :])
                    # bounce through DRAM to reshape [2w+g, n] -> [g, w, n]
                    rdram = dramp.tile([32, 512], BF, name="rdram", bufs=2)
                    nc.sync.dma_start(
                        out=rdram[0 : 2 * NW, :], in_=r32b[0 : 2 * NW, :]
                    )
                    NWH = NW // 2
                    r2a = []
                    for h in range(2):
                        r2h = rp.tile([2, NWH, 512], BF, name="r2h")
                        nc.sync.dma_start(
                            out=r2h,
                            in_=rdram[2 * NWH * h : 2 * NWH * (h + 1), :].rearrange(
                                "(w g) n -> g w n", g=2
                            ),
                        )
                        r2a.append(r2h)
                    state[hp] = (e, rs, r2a)

                def emit_bcast(hp):
                    e, rs, r2a = state.pop(hp)
                    NWH = NW // 2
                    attn = anp.tile([128, ntok], BF, name="attn")
                    for w in range(NW):
                        sc = scp.tile([128, 512], F32, name="sc")
                        nc.tensor.matmul(
                            sc,
                            lhsT=D2,
                            rhs=r2a[w // NWH][:, w % NWH, :],
                            start=True,
                            stop=True,
                        )
                        nc.vector.scalar_tensor_tensor(
                            out=attn[:, 512 * w : 512 * (w + 1)],
                            in0=e[:, 512 * w : 512 * (w + 1)],
                            scalar=rs,
                            in1=sc,
                            op0=mybir.AluOpType.mult,
                            op1=mybir.AluOpType.mult,
                        )
                    nc.sync.dma_start(out=scratch[hp], in_=attn)

                for hp in range(n_hp + 2):
                    if hp < n_hp:
                        emit_step1(hp)
                    if 1 <= hp <= n_hp:
                        emit_jsum(hp - 1)
                    if hp >= 2:
                        emit_bcast(hp - 2)

            # ---- phase 2: out[n,c] = sum_hp attn_hp[:, chunk].T @ p2_hp ----
            with tc.tile_pool(name="a2p", bufs=3) as a2p, tc.tile_pool(
                name="op", bufs=4, space="PSUM"
            ) as op, tc.tile_pool(name="osp", bufs=3) as osp:
                for i2 in range(NT // 2):
                    osb = osp.tile([128, 2, DIM], F32, name="osb")
                    for j in range(2):
                        i = 2 * i2 + j
                        a2 = a2p.tile([128, n_hp, 128], BF, name="a2")
                        nc.sync.dma_start(
                            out=a2,
                            in_=scratch[:, :, 128 * i : 128 * (i + 1)].rearrange(
                                "h p n -> p h n"
                            ),
                        )
                        po = op.tile([128, 512], F32, name="po")
                        for hpi in range(n_hp):
                            nc.tensor.matmul(
                                po,
                                lhsT=a2[:, hpi, :],
                                rhs=p2_sb[:, hpi, :],
                                start=(hpi == 0),
                                stop=(hpi == n_hp - 1),
                            )
                        nc.scalar.activation(osb[:, j, :], po, func=AF.Copy)
                    nc.sync.dma_start(
                        out=out_p[256 * i2 : 256 * (i2 + 1), :].rearrange(
                            "(a p) c -> p a c", p=128
                        ),
                        in_=osb,
                    )


def fuse_weights(inputs):
    tw = np.asarray(inputs["trans_w"], np.float64)  # [4096, 512]
    tb = np.asarray(inputs["trans_b"], np.float64)  # [4096]
    l0w = np.asarray(inputs["lin0_w"], np.float64)  # [64, 64]
    l0b = np.asarray(inputs["lin0_b"], np.float64)
    l1w = np.asarray(inputs["lin1_w"], np.float64)
    l1b = np.asarray(inputs["lin1_b"], np.float64)
    pw = np.asarray(inputs["proj_w"], np.float64)  # [512, 4096]
    pb = np.asarray(inputs["proj_b"], np.float64)

    tw3 = tw.reshape(HEADS, K, DIM)
    tb2 = tb.reshape(HEADS, K)
    fw = np.einsum("jk,hkc->hjc", l0w, tw3)  # [64, 64, 512]
    fb = l0b[None, :] + np.einsum("jk,hk->hj", l0w, tb2)  # [64, 64]
    pw3 = pw.reshape(DIM, HEADS, K).transpose(1, 0, 2)  # [h, c, j]
    g = np.einsum("hcj,jk->hck", pw3, l1w)  # [64, 512, 64]
    cb = pb + np.einsum("hcj,j->c", pw3, l1b)  # [512]
    return fw, fb, g, cb


def make_core_inputs(x, fw, fb, g, b, gg, n_hp=HPC):
    """Inputs for the core handling batch b, head half gg (heads 32*gg..+32)."""
    ntok = x.shape[1]
    h0 = (HEADS // 2) * gg
    w2 = np.empty((128, n_hp, 4, 128), BF16NP)
    c1 = np.empty((128, n_hp), np.float32)
    p2 = np.empty((128, n_hp, DIM), BF16NP)
    for hp in range(n_hp):
        ha, hb = h0 + 2 * hp, h0 + 2 * hp + 1
        blk = np.concatenate([fw[ha], fw[hb]], axis=0)  # [128 j2, 512 c]
        # w2[ci, hp, cc, j2] = blk[j2, cc*128+ci]
        w2[:, hp, :, :] = blk.reshape(128, 4, 128).transpose(2, 1, 0).astype(BF16NP)
        c1[:, hp] = np.concatenate([fb[ha], fb[hb]]).astype(np.float32)
        # p2[g2*64+k, hp, c] = g[head, c, k]
        p2[0:64, hp, :] = g[ha].T.astype(BF16NP)
        p2[64:128, hp, :] = g[hb].T.astype(BF16NP)
    d2 = np.zeros((2, 128), BF16NP)
    d2[0, 0:64] = 1.0
    d2[1, 64:128] = 1.0
    return {
        "x_in": np.ascontiguousarray(x[b], np.float32),
        "w2": w2,
        "c1": c1,
        "p2": p2,
        "d2": d2,
    }


_NC_CACHE = None
LAST_RESULTS = None


def kernel(**inputs):
    global _NC_CACHE, LAST_RESULTS
    import os
    from concourse.bass_utils import run_bass_kernel_spmd

    x = np.asarray(inputs["x"], np.float32)
    fw, fb, g, cb = fuse_weights(inputs)

    if _NC_CACHE is None:
        _NC_CACHE = build_bass()
    nc = _NC_CACHE

    in_maps = []
    for c in range(NCORES):
        b, gg = c // 2, c % 2
        in_maps.append(make_core_inputs(x, fw, fb, g, b, gg))

    trace = os.environ.get("MEA_TRACE", "0") == "1"
    res = run_bass_kernel_spmd(nc, in_maps, list(range(NCORES)), trace=trace)
    LAST_RESULTS = res

    out = np.empty((B, NTOK, DIM), np.float32)
    cbf = cb.astype(np.float32)
    for b in range(B):
        out[b] = res.results[2 * b]["out_p"] + res.results[2 * b + 1]["out_p"]
        out[b] += cbf[None, :]
    return out


# revision 59
# speedup vs baseline: 140.7466x; 1.1501x over previous
"""MultiHeadExternalAttention Trainium2 kernel.

Math (reference):
  h = x @ trans_w.T + trans_b            [B,N,4096] -> heads [B,64,N,64]
  a = h @ lin0_w.T + lin0_b              per-head [B,64,N,64]
  a = softmax(a, axis=N)
  a = a / (1e-10 + a.sum(-1, keepdims))  double norm over j
  o = a @ lin1_w.T + lin1_b
  out = o (merged heads) @ proj_w.T + proj_b

h is only consumed through lin0, and lin1 feeds straight into proj, so both
tiny linears fold into the big matmuls on the host:
  logits[b,h,n,j] = x[b,n,:] @ fw[h,j,:] + fb[h,j]     fw = lin0_w @ trans_w_h
  out[b,n,c]     += attn[b,h,n,k] * g[h,c,k]           g  = proj_w_h @ lin1_w
  out[b,n,c]     += cb[c]                               (host)

Sharding: 8 cores = 4 batches x 2 head-halves (32 heads = 16 head-pairs per
core). Per (core, head-pair): logits computed in [j2=128, n] layout (j2 = two
heads' 64 lin0-outputs stacked) so softmax over n is a free-dim op; attn is
written back through a j-sum matmul (shifted-mask trick -> [2w+g, n] psum
rows), reciprocal, broadcast matmul, and one fused scalar_tensor_tensor.
Normalized attn (bf16) goes to a DRAM scratch; phase 2 re-reads it chunk-wise
as matmul lhsT against the folded projection weights, accumulating all 16
head-pairs in PSUM. Host sums the two head-half partials per batch + bias.
"""

import sys

if "/opt/trn_rl_repo" not in sys.path:
    sys.path.insert(0, "/opt/trn_rl_repo")

import numpy as np
import ml_dtypes

import concourse.bass as bass
import concourse.bacc as bacc
import concourse.mybir as mybir
import concourse.tile as tile
from concourse.masks import make_identity

BF16NP = ml_dtypes.bfloat16
F32 = mybir.dt.float32
F32R = mybir.dt.float32r
BF = mybir.dt.bfloat16
AF = mybir.ActivationFunctionType

DIM = 512
HEADS = 64
K = 64
B = 4
NTOK = 8192
NCORES = 8
HPC = 16  # head pairs per core


def build_bass(ntok=NTOK, n_hp=HPC, reps=1):
    nc = bacc.Bacc()
    x_in = nc.dram_tensor("x_in", [ntok, DIM], F32, kind="ExternalInput")
    w2 = nc.dram_tensor("w2", [128, n_hp, 4, 128], BF, kind="ExternalInput")
    c1 = nc.dram_tensor("c1", [128, n_hp], F32, kind="ExternalInput")
    p2 = nc.dram_tensor("p2", [128, n_hp, DIM], BF, kind="ExternalInput")
    d2 = nc.dram_tensor("d2", [128, 128], BF, kind="ExternalInput")
    out_p = nc.dram_tensor("out_p", [ntok, DIM], F32, kind="ExternalOutput")

    NT = ntok // 128
    NW = ntok // 512
    NCH = ntok // 1024

    with tile.TileContext(nc) as tc:
        with tc.tile_pool(name="const", bufs=1) as const, tc.tile_pool(
            name="dramp", bufs=1, space="DRAM"
        ) as dramp:
            scratch = dramp.tile([max(n_hp - 2, 1), 128, ntok], BF)

            w2_sb = const.tile([128, n_hp, 4, 128], BF)
            nc.scalar.dma_start(out=w2_sb, in_=w2[:])
            c1_sb = const.tile([128, n_hp], F32)
            nc.scalar.dma_start(out=c1_sb, in_=c1[:])
            p2_sb = const.tile([128, n_hp, DIM], BF)
            nc.scalar.dma_start(out=p2_sb, in_=p2[:])

            xT = [const.tile([128, ntok], BF, name=f"xT{i}") for i in range(4)]
            # D2: broadcast mask, ones block per head group, replicated at
            # partitions 32q (host constant) so bcast lhsT/rhs bases align.
            D2 = const.tile([128, 128], BF)
            nc.scalar.dma_start(out=D2, in_=d2[:])

            identb = const.tile([128, 128], BF)
            make_identity(nc, identb)

            for _rep in range(reps):
                run_pipeline(
                    nc, tc, x_in, out_p, scratch, w2_sb, c1_sb, p2_sb, xT, D2,
                    identb, dramp, ntok, n_hp,
                )
    nc.finalize()
    return nc


def run_pipeline(
    nc, tc, x_in, out_p, scratch, w2_sb, c1_sb, p2_sb, xT, D2, identb, dramp, ntok,
    n_hp,
):
    NT = ntok // 128
    NW = ntok // 512
    NCH = ntok // 1024
    if True:
        if True:
            # ---- x load + bf16 cast + PE-transpose into xT[cc][c, n] ----
            with tc.tile_pool(name="xload", bufs=3) as xload, tc.tile_pool(
                name="xcast", bufs=3
            ) as xcast, tc.tile_pool(name="xtp", bufs=4, space="PSUM") as xtp:
                batches = [(0, 1), (1, 1), (2, 2)]
                t0 = 4
                while t0 < NT:
                    batches.append((t0, min(4, NT - t0)))
                    t0 += 4
                for t0, nb in batches:
                    xf = xload.tile([128, 4, DIM], F32, name="xf")
                    nc.sync.dma_start(
                        out=xf[:, 0:nb, :],
                        in_=x_in[128 * t0 : 128 * (t0 + nb), :].rearrange(
                            "(a p) c -> p a c", p=128
                        ),
                    )
                    xb = xcast.tile([128, 4, DIM], BF, name="xb")
                    nc.vector.tensor_copy(xb[:, 0:nb, :], xf[:, 0:nb, :])
                    for a in range(nb):
                        t = t0 + a
                        for cc in range(4):
                            xps = xtp.tile([128, 128], BF, name="xps")
                            nc.tensor.transpose(
                                xps, xb[:, a, 128 * cc : 128 * (cc + 1)], identb
                            )
                            nc.scalar.activation(
                                xT[cc][:, 128 * t : 128 * (t + 1)], xps, func=AF.Copy
                            )

            # ---- phase 1 + phase 2, one pool scope so the tail interleaves:
            # the last two attn slabs never touch DRAM (read from SBUF), and
            # the first phase-2 chunks hide the final bcast/STT drain.
            ns = max(n_hp - 2, 0)
            tail_attn = {}
            with tc.tile_pool(name="anp", bufs=2) as anp, tc.tile_pool(
                name="p1p", bufs=3, space="PSUM"
              ) as p1p, tc.tile_pool(
                name="jsp", bufs=1, space="PSUM"
              ) as jsp, tc.tile_pool(
                name="scp", bufs=2, space="PSUM"
              ) as scp, tc.tile_pool(name="ep", bufs=3) as ep, tc.tile_pool(
                name="rp", bufs=2
              ) as rp, tc.tile_pool(name="tmp", bufs=2) as tmp, tc.tile_pool(
                name="small", bufs=3
              ) as small, tc.tile_pool(name="a2p", bufs=2) as a2p, tc.tile_pool(
                name="op", bufs=2, space="PSUM"
              ) as op, tc.tile_pool(name="osp", bufs=3) as osp:
                state = {}

                def gen_step1(hp):
                    e = ep.tile([128, ntok], BF, name="e")
                    scol = small.tile([128, NW], F32, name="scol")
                    state[hp] = [e, None, None]
                    for t8 in range(NW):
                        p1 = p1p.tile([128, 512], F32, name="p1")
                        base = 512 * t8
                        for cc in range(4):
                            nc.tensor.matmul(
                                p1,
                                lhsT=w2_sb[:, hp, cc, :],
                                rhs=xT[cc][:, base : base + 512],
                                start=(cc == 0),
                                stop=(cc == 3),
                            )
                        nc.scalar.activation(
                            e[:, base : base + 512],
                            p1,
                            func=AF.Exp,
                            bias=c1_sb[:, hp : hp + 1],
                            scale=1.0,
                            accum_out=scol[:, t8 : t8 + 1],
                        )
                        if t8 % 4 == 3:
                            yield
                    s1 = small.tile([128, 1], F32, name="s1")
                    nc.vector.reduce_sum(s1, scol, axis=mybir.AxisListType.X)
                    rs = small.tile([128, 1], F32, name="rs")
                    nc.vector.reciprocal(rs, s1)
                    Tmh = tmp.tile([128, 66], BF, name="Tmh")
                    nc.vector.memset(Tmh, 0.0)
                    nc.vector.tensor_copy(Tmh[0:64, 32:33], rs[0:64, :])
                    nc.vector.tensor_copy(Tmh[64:128, 33:34], rs[64:128, :])
                    state[hp][1] = rs
                    state[hp][2] = Tmh

                def emit_jsum(hp):
                    e, rs, Tmh = state[hp]
                    assert Tmh is not None
                    js = jsp.tile([32, 512], F32, name="js")
                    for w in range(NW):
                        nc.tensor.matmul(
                            js,
                            lhsT=Tmh[:, 32 - 2 * w : 64 - 2 * w],
                            rhs=e[:, 512 * w : 512 * (w + 1)],
                            start=(w == 0),
                            stop=(w == NW - 1),
                        )
                    r32 = rp.tile([32, 512], F32, name="r32")
                    nc.vector.reciprocal(r32[0 : 2 * NW, :], js[0 : 2 * NW, :])
                    r32b = rp.tile([32, 512], BF, name="r32b")
                    nc.vector.tensor_copy(r32b[0 : 2 * NW, :], r32[0 : 2 * NW, :])
                    # bounce through DRAM to reshape [2w+g, n] -> [g, w, n]
                    rdram = dramp.tile([32, 512], BF, name="rdram", bufs=2)
                    nc.sync.dma_start(
                        out=rdram[0 : 2 * NW, :], in_=r32b[0 : 2 * NW, :]
                    )
                    # pack window w at partitions 32*(w%Q)+{0,1}, free slot w//Q
                    Q = min(4, NW)
                    NS = NW // Q
                    r2a = rp.tile([128, NS, 512], BF, name="r2a")
                    rv = rdram[0 : 2 * NW, :].rearrange(
                        "(s q g) n -> q g s n", q=Q, g=2
                    )
                    for q in range(Q):
                        nc.sync.dma_start(
                            out=r2a[32 * q : 32 * q + 2, :, :], in_=rv[q]
                        )
                    state[hp] = [e, rs, r2a]

                def gen_bcast(hp):
                    e, rs, r2a = state.pop(hp)
                    Q = min(4, NW)
                    attn = anp.tile([128, ntok], BF, name="attn")
                    if hp >= ns:
                        tail_attn[hp] = attn
                    for w in range(NW):
                        q = w % Q
                        sc = scp.tile([128, 512], F32, name="sc")
                        nc.tensor.matmul(
                            sc,
                            lhsT=D2[32 * q : 32 * q + 2, :],
                            rhs=r2a[32 * q : 32 * q + 2, w // Q, :],
                            start=True,
                            stop=True,
                            tile_position=(32 * q, 0),
                        )
                        nc.vector.scalar_tensor_tensor(
                            out=attn[:, 512 * w : 512 * (w + 1)],
                            in0=e[:, 512 * w : 512 * (w + 1)],
                            scalar=rs,
                            in1=sc,
                            op0=mybir.AluOpType.mult,
                            op1=mybir.AluOpType.mult,
                        )
                        if w % 4 == 3:
                            yield
                    if hp < ns:
                        nc.sync.dma_start(out=scratch[hp], in_=attn)

                # phase 2: out[n,c] = sum_hp attn_hp[:, chunk].T @ p2_hp
                def gen_phase2():
                    for i in range(NT):
                        if ns > 0:
                            a2 = a2p.tile([128, ns, 128], BF, name="a2")
                            nc.sync.dma_start(
                                out=a2,
                                in_=scratch[
                                    0:ns, :, 128 * i : 128 * (i + 1)
                                ].rearrange("h p n -> p h n"),
                            )
                        po = op.tile([128, 512], F32, name="po")
                        for hpi in range(n_hp):
                            if hpi < ns:
                                lhsT = a2[:, hpi, :]
                            else:
                                lhsT = tail_attn[hpi][:, 128 * i : 128 * (i + 1)]
                            nc.tensor.matmul(
                                po,
                                lhsT=lhsT,
                                rhs=p2_sb[:, hpi, :],
                                start=(hpi == 0),
                                stop=(hpi == n_hp - 1),
                            )
                        osb = osp.tile([128, DIM], F32, name="osb")
                        nc.scalar.activation(osb, po, func=AF.Copy)
                        nc.sync.dma_start(
                            out=out_p[128 * i : 128 * (i + 1), :], in_=osb
                        )
                        yield

                p2g = gen_phase2()
                for hp in range(n_hp + 2):
                    if 1 <= hp <= n_hp:
                        emit_jsum(hp - 1)
                    s1g = gen_step1(hp) if hp < n_hp else None
                    bcg = gen_bcast(hp - 2) if hp >= 2 else None
                    tail = hp == n_hp + 1
                    while s1g is not None or bcg is not None:
                        if s1g is not None and next(s1g, "END") == "END":
                            s1g = None
                        if bcg is not None and next(bcg, "END") == "END":
                            bcg = None
                        if tail:
                            next(p2g, None)
                    if hp == 0 and n_hp == 1:
                        pass
                for _ in p2g:
                    pass
            tail_attn.clear()


def fuse_weights(inputs):
    tw = np.asarray(inputs["trans_w"], np.float64)  # [4096, 512]
    tb = np.asarray(inputs["trans_b"], np.float64)  # [4096]
    l0w = np.asarray(inputs["lin0_w"], np.float64)  # [64, 64]
    l0b = np.asarray(inputs["lin0_b"], np.float64)
    l1w = np.asarray(inputs["lin1_w"], np.float64)
    l1b = np.asarray(inputs["lin1_b"], np.float64)
    pw = np.asarray(inputs["proj_w"], np.float64)  # [512, 4096]
    pb = np.asarray(inputs["proj_b"], np.float64)

    tw3 = tw.reshape(HEADS, K, DIM)
    tb2 = tb.reshape(HEADS, K)
    fw = np.einsum("jk,hkc->hjc", l0w, tw3)  # [64, 64, 512]
    fb = l0b[None, :] + np.einsum("jk,hk->hj", l0w, tb2)  # [64, 64]
    pw3 = pw.reshape(DIM, HEADS, K).transpose(1, 0, 2)  # [h, c, j]
    g = np.einsum("hcj,jk->hck", pw3, l1w)  # [64, 512, 64]
    cb = pb + np.einsum("hcj,j->c", pw3, l1b)  # [512]
    return fw, fb, g, cb


def make_core_inputs(x, fw, fb, g, b, gg, n_hp=HPC):
    """Inputs for the core handling batch b, head half gg (heads 32*gg..+32)."""
    ntok = x.shape[1]
    h0 = (HEADS // 2) * gg
    w2 = np.empty((128, n_hp, 4, 128), BF16NP)
    c1 = np.empty((128, n_hp), np.float32)
    p2 = np.empty((128, n_hp, DIM), BF16NP)
    for hp in range(n_hp):
        ha, hb = h0 + 2 * hp, h0 + 2 * hp + 1
        blk = np.concatenate([fw[ha], fw[hb]], axis=0)  # [128 j2, 512 c]
        # w2[ci, hp, cc, j2] = blk[j2, cc*128+ci]
        w2[:, hp, :, :] = blk.reshape(128, 4, 128).transpose(2, 1, 0).astype(BF16NP)
        c1[:, hp] = np.concatenate([fb[ha], fb[hb]]).astype(np.float32)
        # p2[g2*64+k, hp, c] = g[head, c, k]
        p2[0:64, hp, :] = g[ha].T.astype(BF16NP)
        p2[64:128, hp, :] = g[hb].T.astype(BF16NP)
    d2 = np.zeros((128, 128), BF16NP)
    for q in range(4):
        d2[32 * q + 0, 0:64] = 1.0
        d2[32 * q + 1, 64:128] = 1.0
    return {
        "x_in": np.ascontiguousarray(x[b], np.float32),
        "w2": w2,
        "c1": c1,
        "p2": p2,
        "d2": d2,
    }


_NC_CACHE = None
LAST_RESULTS = None


def kernel(**inputs):
    global _NC_CACHE, LAST_RESULTS
    import os
    from concourse.bass_utils import run_bass_kernel_spmd

    x = np.asarray(inputs["x"], np.float32)
    fw, fb, g, cb = fuse_weights(inputs)

    if _NC_CACHE is None:
        _NC_CACHE = build_bass()
    nc = _NC_CACHE

    in_maps = []
    for c in range(NCORES):
        b, gg = c // 2, c % 2
        in_maps.append(make_core_inputs(x, fw, fb, g, b, gg))

    trace = os.environ.get("MEA_TRACE", "0") == "1"
    res = run_bass_kernel_spmd(nc, in_maps, list(range(NCORES)), trace=trace)
    LAST_RESULTS = res

    out = np.empty((B, NTOK, DIM), np.float32)
    cbf = cb.astype(np.float32)
    for b in range(B):
        out[b] = res.results[2 * b]["out_p"] + res.results[2 * b + 1]["out_p"]
        out[b] += cbf[None, :]
    return out
